# revision 1
# baseline (speedup 1.0000x reference)
"""Bass/Trainium2 kernel for a 2-layer LSTM language model.

Model (see problem reference): emb = inputs @ emb_w; 2 stacked LSTM layers
(nn=1536) scanned over T=256; logits = h1 @ out_w + out_b.

Sharding: tensor-parallel over the 4*nn gate dimension across 8 cores.
Core k owns nn-slice [192k, 192k+192) of both layers (the f/i/o/g columns
for that slice, re-packed contiguously), keeps c local, and the per-step
hidden chunks are AllGathered in bf16 feature-major layout ([192,64] per
rank -> [1536,64]). The embedding contribution to layer-0 gates is folded
into one big precomputed GEMM: xpart = inputs @ (emb_w @ W0x_slice).
The vocab projection is sharded over vocab (64 cols/core) and computed
inside the time loop as PE filler work.
"""

import sys

sys.path.insert(0, "/opt/trn_rl_repo")

import numpy as np
import ml_dtypes

import concourse.bass as bass
import concourse.mybir as mybir
import concourse.tile as tile
from concourse import bass_utils

BF16 = mybir.dt.bfloat16
F32 = mybir.dt.float32
AF = mybir.ActivationFunctionType

T, B, V, E, NN = 256, 64, 512, 256, 1536
NCORES = 8
S = NN // NCORES          # 192  nn slice per core
G4 = 4 * S                # 768  gate cols per core
VS = V // NCORES          # 64   vocab slice per core
KT0 = NN // 128           # 12   k-tiles of h
KTE = V // 128            # 4    k-tiles of vocab (embedding GEMM contraction)

_MAXW = 1  # walrus sync-wait limit per instruction in this toolchain


def _split_sync_waits(nc, maxw=_MAXW):
    """walrus codegen rejects instructions with >maxw sync waits; move the
    overflow onto no-ops inserted just before (same engine, program order)."""
    for bb in nc.main_func.blocks:
        insts = bb.instructions
        i = 0
        while i < len(insts):
            inst = insts[i]
            si = inst.sync_info
            if si is not None and len(si.on_wait) > maxw:
                waits = list(si.on_wait)
                overflow, keep = waits[:-maxw], waits[-maxw:]
                inst.sync_info = mybir.SyncInfo(
                    on_wait=keep, on_update=list(si.on_update)
                )
                pos = i
                for j in range(0, len(overflow), maxw):
                    nop = mybir.InstNoOp(
                        name=nc.get_next_instruction_name(), ins=[], outs=[]
                    )
                    nop.engine = inst.engine
                    nop.sync_info = mybir.SyncInfo(
                        on_wait=overflow[j : j + maxw], on_update=[]
                    )
                    nc.register_instruction(nop, overwrite=True)
                    insts.insert(pos, nop)
                    pos += 1
                    i += 1
            i += 1


def build_program(t_steps=T, use_coll=True):
    ntok = t_steps * B
    mtok = ntok // 128

    nc = bass.Bass(
        "TRN2", target_bir_lowering=False, debug=False, num_devices=NCORES
    )

    # ---- kernel I/O (per core) ----
    inputsT = nc.dram_tensor("inputsT", [V, ntok], BF16, kind="ExternalInput").ap()
    emb_wT = nc.dram_tensor("emb_wT", [E, V], BF16, kind="ExternalInput").ap()
    w0x = nc.dram_tensor("w0x", [E, G4], BF16, kind="ExternalInput").ap()
    w0h = nc.dram_tensor("w0h", [NN, G4], BF16, kind="ExternalInput").ap()
    w1x = nc.dram_tensor("w1x", [NN, G4], BF16, kind="ExternalInput").ap()
    w1h = nc.dram_tensor("w1h", [NN, G4], BF16, kind="ExternalInput").ap()
    outw = nc.dram_tensor("outw", [NN, VS], BF16, kind="ExternalInput").ap()
    b0rep = nc.dram_tensor("b0rep", [128, G4], F32, kind="ExternalInput").ap()
    b1rep = nc.dram_tensor("b1rep", [64, G4], BF16, kind="ExternalInput").ap()
    outbrep = nc.dram_tensor("outbrep", [64, VS], F32, kind="ExternalInput").ap()
    h0T0 = nc.dram_tensor("h0T0", [NN, B], BF16, kind="ExternalInput").ap()
    h1T0 = nc.dram_tensor("h1T0", [NN, B], BF16, kind="ExternalInput").ap()
    c0in = nc.dram_tensor("c0in", [B, S], F32, kind="ExternalInput").ap()
    c1in = nc.dram_tensor("c1in", [B, S], F32, kind="ExternalInput").ap()
    ident = nc.dram_tensor("ident", [64, 64], F32, kind="ExternalInput").ap()
    identb = nc.dram_tensor("identb", [64, 64], BF16, kind="ExternalInput").ap()

    logits = nc.dram_tensor(
        "logits", [t_steps, B, VS], F32, kind="ExternalOutput"
    ).ap()

    rg = [list(range(NCORES))]

    with tile.TileContext(nc) as tc:
        with (
            tc.tile_pool(name="const", bufs=1) as const,
            tc.tile_pool(name="dram", bufs=1, space="DRAM") as dram,
            tc.tile_pool(name="ring", bufs=3, space="DRAM") as ring,
            tc.tile_pool(name="sb", bufs=3) as sb,
        ):
            # ---- persistent SBUF: weights, biases, state ----
            w0h_sb = const.tile([128, KT0 * G4], BF16, tag="w0h")
            nc.sync.dma_start(
                w0h_sb[:].rearrange("p (j n) -> p j n", j=KT0),
                w0h.rearrange("(j p) n -> p j n", p=128),
            )
            w1h_sb = const.tile([128, KT0 * G4], BF16, tag="w1h")
            nc.sync.dma_start(
                w1h_sb[:].rearrange("p (j n) -> p j n", j=KT0),
                w1h.rearrange("(j p) n -> p j n", p=128),
            )
            w1x_sb = const.tile([128, KT0 * G4], BF16, tag="w1x")
            nc.sync.dma_start(
                w1x_sb[:].rearrange("p (j n) -> p j n", j=KT0),
                w1x.rearrange("(j p) n -> p j n", p=128),
            )
            outw_sb = const.tile([128, KT0 * VS], BF16, tag="outw")
            nc.sync.dma_start(
                outw_sb[:].rearrange("p (j n) -> p j n", j=KT0),
                outw.rearrange("(j p) n -> p j n", p=128),
            )
            b0rep_sb = const.tile([128, G4], F32, tag="b0rep")
            nc.sync.dma_start(b0rep_sb[:], b0rep[:])
            b1rep_sb = const.tile([64, G4], BF16, tag="b1rep")
            nc.sync.dma_start(b1rep_sb[:], b1rep[:])
            outbrep_sb = const.tile([64, VS], F32, tag="outbrep")
            nc.sync.dma_start(outbrep_sb[:], outbrep[:])
            ident_sb = const.tile([64, 64], F32, tag="ident")
            nc.sync.dma_start(ident_sb[:], ident[:])
            identb_sb = const.tile([64, 64], BF16, tag="identb")
            nc.sync.dma_start(identb_sb[:], identb[:])
            c0_sb = const.tile([B, S], F32, tag="c0")
            nc.sync.dma_start(c0_sb[:], c0in[:])
            c1_sb = const.tile([B, S], F32, tag="c1")
            nc.sync.dma_start(c1_sb[:], c1in[:])

            # ---- phase 0: P = emb_w @ w0x  -> pT_sb [128, 4*G4] bf16 ----
            pT_sb = const.tile([128, KTE * G4], BF16, tag="pT")
            with (
                tc.tile_pool(name="ph0sb", bufs=2) as ph0sb,
                tc.tile_pool(name="ph0ps", bufs=2, space="PSUM") as ph0ps,
            ):
                for m in range(KTE):  # vocab m-tiles of P
                    lt = ph0sb.tile([128, 2 * 128], BF16, tag="ph0l")
                    nc.sync.dma_start(
                        lt[:].rearrange("p (k q) -> p k q", k=2),
                        emb_wT[:, 128 * m : 128 * (m + 1)].rearrange(
                            "(k p) q -> p k q", p=128
                        ),
                    )
                    rt = ph0sb.tile([128, 2 * G4], BF16, tag="ph0r")
                    nc.sync.dma_start(
                        rt[:].rearrange("p (k n) -> p k n", k=2),
                        w0x.rearrange("(k p) n -> p k n", p=128),
                    )
                    pp = ph0ps.tile([128, G4], F32, tag="ph0ps")
                    for k in range(2):
                        for n0, n1 in ((0, 512), (512, G4)):
                            nc.tensor.matmul(
                                pp[:, n0:n1],
                                lt[:, 128 * k : 128 * (k + 1)],
                                rt[:, G4 * k + n0 : G4 * k + n1],
                                start=(k == 0),
                                stop=(k == 1),
                            )
                    nc.vector.tensor_copy(pT_sb[:, G4 * m : G4 * (m + 1)], pp[:])

            # ---- phase 1: xpart = inputsT.T @ P + b0  -> DRAM [ntok, G4] f32
            xpart = dram.tile([ntok, G4], BF16, tag="xpart")
            with (
                tc.tile_pool(name="ph1sb", bufs=3) as ph1sb,
                tc.tile_pool(name="ph1ps", bufs=2, space="PSUM") as ph1ps,
            ):
                for m in range(mtok):
                    lt = ph1sb.tile([128, KTE * 128], BF16, tag="ph1l")
                    nc.sync.dma_start(
                        lt[:].rearrange("p (k q) -> p k q", k=KTE),
                        inputsT[:, 128 * m : 128 * (m + 1)].rearrange(
                            "(k p) q -> p k q", p=128
                        ),
                    )
                    pp = ph1ps.tile([128, G4], F32, tag="ph1ps")
                    for k in range(KTE):
                        for n0, n1 in ((0, 512), (512, G4)):
                            nc.tensor.matmul(
                                pp[:, n0:n1],
                                lt[:, 128 * k : 128 * (k + 1)],
                                pT_sb[:, G4 * k + n0 : G4 * k + n1],
                                start=(k == 0),
                                stop=(k == KTE - 1),
                            )
                    xf = ph1sb.tile([128, G4], BF16, tag="ph1o")
                    nc.vector.tensor_add(xf[:], pp[:], b0rep_sb[:])
                    nc.sync.dma_start(xpart[128 * m : 128 * (m + 1), :], xf[:])

            # ---- initial hidden state tiles (feature-major [128, 12*64]) ----
            def load_hT(src, tag):
                t = sb.tile([128, KT0 * B], BF16, tag=tag)
                nc.sync.dma_start(
                    t[:].rearrange("p (j b) -> p j b", j=KT0),
                    src.rearrange("(j p) b -> p j b", p=128),
                )
                return t

            h0_prev = load_hT(h0T0, "h0")
            h1_prev = load_hT(h1T0, "h1")

            with (
                tc.tile_pool(name="psA", bufs=2, space="PSUM") as psA,
                tc.tile_pool(name="psC", bufs=1, space="PSUM") as psC,
                tc.tile_pool(name="psB", bufs=1, space="PSUM") as psB,
            ):
                # PSUM banks: lin0 2x2 + lin1 2 + lout 1 + tr 1 = 8

                def gate_chain(lin_ps, c_sb, layer):
                    """sigmoid/tanh gates, c update -> h_new [64,S] f32."""
                    act = sb.tile([B, G4], F32, tag=f"act{layer}")
                    nc.scalar.activation(
                        act[:, 0 : 3 * S], lin_ps[:, 0 : 3 * S], AF.Sigmoid
                    )
                    nc.scalar.activation(
                        act[:, 3 * S : G4], lin_ps[:, 3 * S : G4], AF.Tanh
                    )
                    fc = sb.tile([B, S], F32, tag=f"fc{layer}")
                    nc.vector.tensor_mul(fc[:], act[:, 0:S], c_sb[:])
                    ig = sb.tile([B, S], F32, tag=f"ig{layer}")
                    nc.vector.tensor_mul(
                        ig[:], act[:, S : 2 * S], act[:, 3 * S : G4]
                    )
                    nc.vector.tensor_add(c_sb[:], fc[:], ig[:])
                    th = sb.tile([B, S], F32, tag=f"th{layer}")
                    nc.scalar.activation(th[:], c_sb[:], AF.Tanh)
                    hn = sb.tile([B, S], F32, tag=f"hn{layer}")
                    nc.vector.tensor_mul(hn[:], act[:, 2 * S : 3 * S], th[:])
                    return hn

                def transpose_out(hn, ag_in, layer):
                    """[64,S] f32 -> bf16 feature-major chunks -> ag_in DRAM."""
                    tp = psB.tile([128, B], F32, tag="tr")
                    nc.tensor.transpose(tp[:], hn[:, 0:128], ident_sb[:])
                    tb = sb.tile([128, B], BF16, tag=f"trc{layer}")
                    nc.vector.tensor_copy(tb[:], tp[:])
                    nc.sync.dma_start(ag_in[0:128, :], tb[:])
                    tp2 = psB.tile([64, B], F32, tag="tr")
                    nc.tensor.transpose(tp2[:], hn[:, 128:S], ident_sb[:])
                    tb2 = sb.tile([64, B], BF16, tag=f"tsc{layer}")
                    nc.vector.tensor_copy(tb2[:], tp2[:])
                    nc.sync.dma_start(ag_in[128:S, :], tb2[:])

                def vocab_proj(h_tile, t_out):
                    lo = psB.tile([B, VS], F32, tag="lout")
                    for j in range(KT0):
                        nc.tensor.matmul(
                            lo[:],
                            h_tile[:, B * j : B * (j + 1)],
                            outw_sb[:, VS * j : VS * (j + 1)],
                            start=(j == 0),
                            stop=(j == KT0 - 1),
                        )
                    lsb = sb.tile([B, VS], F32, tag="lsb")
                    nc.vector.tensor_add(lsb[:], lo[:], outbrep_sb[:])
                    nc.sync.dma_start(logits[t_out], lsb[:])

                lout_prev = None  # h1 tile of step t-1, pending vocab proj

                for t in range(t_steps):
                    # --- A: layer-0 gate matmuls (needs h0_prev) ---
                    xp = sb.tile([B, G4], BF16, tag="xp")
                    nc.sync.dma_start(xp[:], xpart[B * t : B * (t + 1), :])
                    lin0 = psA.tile([B, G4], F32, tag="lin0")
                    for n0, n1 in ((0, 512), (512, G4)):
                        nc.tensor.matmul(
                            lin0[:, n0:n1], identb_sb[:], xp[:, n0:n1],
                            start=True, stop=False,
                        )
                        for j in range(KT0):
                            nc.tensor.matmul(
                                lin0[:, n0:n1],
                                h0_prev[:, B * j : B * (j + 1)],
                                w0h_sb[:, G4 * j + n0 : G4 * j + n1],
                                start=False, stop=(j == KT0 - 1),
                            )

                    # --- B: vocab projection of step t-1 (needs h1_prev) ---
                    if lout_prev is not None:
                        vocab_proj(lout_prev, t - 1)

                    # --- C: layer-1 h-part matmuls (needs h1_prev) ---
                    lin1 = psC.tile([B, G4], F32, tag="lin1")
                    for n0, n1 in ((0, 512), (512, G4)):
                        nc.tensor.matmul(
                            lin1[:, n0:n1], identb_sb[:], b1rep_sb[:, n0:n1],
                            start=True, stop=False,
                        )
                        for j in range(KT0):
                            nc.tensor.matmul(
                                lin1[:, n0:n1],
                                h1_prev[:, B * j : B * (j + 1)],
                                w1h_sb[:, G4 * j + n0 : G4 * j + n1],
                                start=False, stop=False,
                            )

                    # --- D: layer-0 gates, AG0 ---
                    h0n = gate_chain(lin0, c0_sb, 0)
                    ag0_in = ring.tile([S, B], BF16, tag="ag0i")
                    transpose_out(h0n, ag0_in, 0)
                    ag0_out = ring.tile([NN, B], BF16, tag="ag0o")
                    if use_coll:
                        nc.gpsimd.collective_compute(
                            "AllGather", mybir.AluOpType.bypass,
                            replica_groups=rg,
                            ins=[ag0_in.opt()], outs=[ag0_out.opt()],
                        )
                    else:
                        nc.sync.dma_start(ag0_out[0:S, :], ag0_in[:])
                    h0_cur = load_hT(ag0_out[:], "h0")

                    # --- E: layer-1 x-part matmuls (needs h0_cur) ---
                    for n0, n1 in ((0, 512), (512, G4)):
                        for j in range(KT0):
                            nc.tensor.matmul(
                                lin1[:, n0:n1],
                                h0_cur[:, B * j : B * (j + 1)],
                                w1x_sb[:, G4 * j + n0 : G4 * j + n1],
                                start=False, stop=(j == KT0 - 1),
                            )

                    # --- F: layer-1 gates, AG1 ---
                    h1n = gate_chain(lin1, c1_sb, 1)
                    ag1_in = ring.tile([S, B], BF16, tag="ag1i")
                    transpose_out(h1n, ag1_in, 1)
                    ag1_out = ring.tile([NN, B], BF16, tag="ag1o")
                    if use_coll:
                        nc.gpsimd.collective_compute(
                            "AllGather", mybir.AluOpType.bypass,
                            replica_groups=rg,
                            ins=[ag1_in.opt()], outs=[ag1_out.opt()],
                        )
                    else:
                        nc.sync.dma_start(ag1_out[0:S, :], ag1_in[:])
                    h1_cur = load_hT(ag1_out[:], "h1")

                    h0_prev, h1_prev, lout_prev = h0_cur, h1_cur, h1_cur

                # tail: vocab projection for the last step
                vocab_proj(lout_prev, t_steps - 1)

    _split_sync_waits(nc)
    return nc


_PROGRAM_CACHE = {}


def _get_program(t_steps=T, use_coll=True):
    key = (t_steps, use_coll)
    if key not in _PROGRAM_CACHE:
        _PROGRAM_CACHE[key] = build_program(t_steps, use_coll)
    return _PROGRAM_CACHE[key]


def make_in_maps(inputs, emb_w, lstm_w0, lstm_b0, lstm_w1, lstm_b1,
                 out_w, out_b, h0, c0, h1, c1, t_steps=T):
    bf16 = ml_dtypes.bfloat16
    f32 = np.float32
    ntok = t_steps * B
    inputsT = np.ascontiguousarray(inputs.reshape(ntok, V).T).astype(bf16)
    emb_wT = np.ascontiguousarray(emb_w.T).astype(bf16)
    ident = np.eye(64, dtype=f32)
    identb = np.eye(64).astype(bf16)

    def gate_cols(w, k):
        # [in, 4*NN] -> per-core [in, 4*S] with [f|i|o|g] blocks
        return np.concatenate(
            [w[:, g * NN + k * S : g * NN + (k + 1) * S] for g in range(4)],
            axis=1,
        )

    in_maps = []
    for k in range(NCORES):
        w0k = gate_cols(lstm_w0, k)
        w1k = gate_cols(lstm_w1, k)
        b0k = gate_cols(lstm_b0[None, :], k)[0]
        b1k = gate_cols(lstm_b1[None, :], k)[0]
        in_maps.append({
            "inputsT": inputsT,
            "emb_wT": emb_wT,
            "w0x": np.ascontiguousarray(w0k[:E]).astype(bf16),
            "w0h": np.ascontiguousarray(w0k[E:]).astype(bf16),
            "w1x": np.ascontiguousarray(w1k[:NN]).astype(bf16),
            "w1h": np.ascontiguousarray(w1k[NN:]).astype(bf16),
            "outw": np.ascontiguousarray(
                out_w[:, k * VS : (k + 1) * VS]
            ).astype(bf16),
            "b0rep": np.broadcast_to(b0k.astype(f32), (128, G4)).copy(),
            "b1rep": np.broadcast_to(b1k, (64, G4)).astype(bf16).copy(),
            "outbrep": np.broadcast_to(
                out_b[k * VS : (k + 1) * VS].astype(f32), (64, VS)
            ).copy(),
            "h0T0": np.ascontiguousarray(h0.T).astype(bf16),
            "h1T0": np.ascontiguousarray(h1.T).astype(bf16),
            "c0in": np.ascontiguousarray(c0[:, k * S : (k + 1) * S]).astype(f32),
            "c1in": np.ascontiguousarray(c1[:, k * S : (k + 1) * S]).astype(f32),
            "ident": ident,
            "identb": identb,
        })
    return in_maps


def kernel(inputs, emb_w, lstm_w0, lstm_b0, lstm_w1, lstm_b1,
           out_w, out_b, h0, c0, h1, c1, _trace=False):
    inputs = np.asarray(inputs, dtype=np.float32)
    t_steps = inputs.shape[0]
    nc = _get_program(t_steps)
    in_maps = make_in_maps(
        inputs,
        np.asarray(emb_w, np.float32), np.asarray(lstm_w0, np.float32),
        np.asarray(lstm_b0, np.float32), np.asarray(lstm_w1, np.float32),
        np.asarray(lstm_b1, np.float32), np.asarray(out_w, np.float32),
        np.asarray(out_b, np.float32), np.asarray(h0, np.float32),
        np.asarray(c0, np.float32), np.asarray(h1, np.float32),
        np.asarray(c1, np.float32), t_steps=t_steps,
    )
    res = bass_utils.run_bass_kernel_spmd(
        nc, in_maps, core_ids=list(range(NCORES)), trace=_trace
    )
    out = np.concatenate(
        [res.results[k]["logits"] for k in range(NCORES)], axis=2
    )
    if _trace:
        kernel.last_results = res
    return out.astype(np.float32)



# revision 2
# speedup vs baseline: 1.1544x; 1.1544x over previous
"""Bass/Trainium2 kernel for a 2-layer LSTM language model.

Model (see problem reference): emb = inputs @ emb_w; 2 stacked LSTM layers
(nn=1536) scanned over T=256; logits = h1 @ out_w + out_b.

Sharding: tensor-parallel over the 4*nn gate dimension across 8 cores.
Core k owns nn-slice [192k, 192k+192) of both layers (the f/i/o/g columns
for that slice, re-packed contiguously), keeps c local, and the per-step
hidden chunks are AllGathered in bf16 feature-major layout ([192,64] per
rank -> [1536,64]). The embedding contribution to layer-0 gates is folded
into one big precomputed GEMM: xpart = inputs @ (emb_w @ W0x_slice).
The vocab projection is sharded over vocab (64 cols/core) and computed
inside the time loop as PE filler work.
"""

import sys

sys.path.insert(0, "/opt/trn_rl_repo")

import numpy as np
import ml_dtypes

import concourse.bass as bass
import concourse.mybir as mybir
import concourse.tile as tile
from concourse import bass_utils

BF16 = mybir.dt.bfloat16
F32 = mybir.dt.float32
AF = mybir.ActivationFunctionType

T, B, V, E, NN = 256, 64, 512, 256, 1536
NCORES = 8
S = NN // NCORES          # 192  nn slice per core
G4 = 4 * S                # 768  gate cols per core
VS = V // NCORES          # 64   vocab slice per core
KT0 = NN // 128           # 12   k-tiles of h
KTE = V // 128            # 4    k-tiles of vocab (embedding GEMM contraction)

_MAXW = 1  # walrus sync-wait limit per instruction in this toolchain


def _split_sync_waits(nc, maxw=_MAXW):
    """walrus codegen rejects instructions with >maxw sync waits; move the
    overflow onto no-ops inserted just before (same engine, program order)."""
    for bb in nc.main_func.blocks:
        insts = bb.instructions
        i = 0
        while i < len(insts):
            inst = insts[i]
            si = inst.sync_info
            if si is not None and len(si.on_wait) > maxw:
                waits = list(si.on_wait)
                overflow, keep = waits[:-maxw], waits[-maxw:]
                inst.sync_info = mybir.SyncInfo(
                    on_wait=keep, on_update=list(si.on_update)
                )
                pos = i
                for j in range(0, len(overflow), maxw):
                    nop = mybir.InstNoOp(
                        name=nc.get_next_instruction_name(), ins=[], outs=[]
                    )
                    nop.engine = inst.engine
                    nop.sync_info = mybir.SyncInfo(
                        on_wait=overflow[j : j + maxw], on_update=[]
                    )
                    nc.register_instruction(nop, overwrite=True)
                    insts.insert(pos, nop)
                    pos += 1
                    i += 1
            i += 1


def build_program(t_steps=T, use_coll=True):
    ntok = t_steps * B
    mtok = ntok // 128

    nc = bass.Bass(
        "TRN2", target_bir_lowering=False, debug=False, num_devices=NCORES
    )

    # ---- kernel I/O (per core) ----
    inputsT = nc.dram_tensor("inputsT", [V, ntok], BF16, kind="ExternalInput").ap()
    emb_wT = nc.dram_tensor("emb_wT", [E, V], BF16, kind="ExternalInput").ap()
    w0x = nc.dram_tensor("w0x", [E, G4], BF16, kind="ExternalInput").ap()
    w0h = nc.dram_tensor("w0h", [NN, G4], BF16, kind="ExternalInput").ap()
    w1x = nc.dram_tensor("w1x", [NN, G4], BF16, kind="ExternalInput").ap()
    w1h = nc.dram_tensor("w1h", [NN, G4], BF16, kind="ExternalInput").ap()
    outw = nc.dram_tensor("outw", [NN, VS], BF16, kind="ExternalInput").ap()
    b0rep = nc.dram_tensor("b0rep", [128, G4], F32, kind="ExternalInput").ap()
    b1rep = nc.dram_tensor("b1rep", [64, G4], BF16, kind="ExternalInput").ap()
    outbrep = nc.dram_tensor("outbrep", [64, VS], F32, kind="ExternalInput").ap()
    h0T0 = nc.dram_tensor("h0T0", [NN, B], BF16, kind="ExternalInput").ap()
    h1T0 = nc.dram_tensor("h1T0", [NN, B], BF16, kind="ExternalInput").ap()
    c0in = nc.dram_tensor("c0in", [B, S], F32, kind="ExternalInput").ap()
    c1in = nc.dram_tensor("c1in", [B, S], F32, kind="ExternalInput").ap()
    ident = nc.dram_tensor("ident", [64, 64], F32, kind="ExternalInput").ap()
    identb = nc.dram_tensor("identb", [64, 64], BF16, kind="ExternalInput").ap()

    logits = nc.dram_tensor(
        "logits", [t_steps, B, VS], F32, kind="ExternalOutput"
    ).ap()

    rg = [list(range(NCORES))]

    with tile.TileContext(nc) as tc:
        with (
            tc.tile_pool(name="const", bufs=1) as const,
            tc.tile_pool(name="dram", bufs=1, space="DRAM") as dram,
            tc.tile_pool(name="ring", bufs=3, space="DRAM") as ring,
            tc.tile_pool(name="sb", bufs=3) as sb,
        ):
            # ---- persistent SBUF: weights, biases, state ----
            w0h_sb = const.tile([128, KT0 * G4], BF16, tag="w0h")
            nc.sync.dma_start(
                w0h_sb[:].rearrange("p (j n) -> p j n", j=KT0),
                w0h.rearrange("(j p) n -> p j n", p=128),
            )
            w1h_sb = const.tile([128, KT0 * G4], BF16, tag="w1h")
            nc.sync.dma_start(
                w1h_sb[:].rearrange("p (j n) -> p j n", j=KT0),
                w1h.rearrange("(j p) n -> p j n", p=128),
            )
            w1x_sb = const.tile([128, KT0 * G4], BF16, tag="w1x")
            nc.sync.dma_start(
                w1x_sb[:].rearrange("p (j n) -> p j n", j=KT0),
                w1x.rearrange("(j p) n -> p j n", p=128),
            )
            outw_sb = const.tile([128, KT0 * VS], BF16, tag="outw")
            nc.sync.dma_start(
                outw_sb[:].rearrange("p (j n) -> p j n", j=KT0),
                outw.rearrange("(j p) n -> p j n", p=128),
            )
            b0rep_sb = const.tile([128, G4], F32, tag="b0rep")
            nc.sync.dma_start(b0rep_sb[:], b0rep[:])
            b1rep_sb = const.tile([64, G4], BF16, tag="b1rep")
            nc.sync.dma_start(b1rep_sb[:], b1rep[:])
            outbrep_sb = const.tile([64, VS], F32, tag="outbrep")
            nc.sync.dma_start(outbrep_sb[:], outbrep[:])
            ident_sb = const.tile([64, 64], F32, tag="ident")
            nc.sync.dma_start(ident_sb[:], ident[:])
            identb_sb = const.tile([64, 64], BF16, tag="identb")
            nc.sync.dma_start(identb_sb[:], identb[:])
            c0_sb = const.tile([B, S], F32, tag="c0")
            nc.sync.dma_start(c0_sb[:], c0in[:])
            c1_sb = const.tile([B, S], F32, tag="c1")
            nc.sync.dma_start(c1_sb[:], c1in[:])

            # ---- phase 0: P = emb_w @ w0x  -> pT_sb [128, 4*G4] bf16 ----
            pT_sb = const.tile([128, KTE * G4], BF16, tag="pT")
            with (
                tc.tile_pool(name="ph0sb", bufs=2) as ph0sb,
                tc.tile_pool(name="ph0ps", bufs=2, space="PSUM") as ph0ps,
            ):
                for m in range(KTE):  # vocab m-tiles of P
                    lt = ph0sb.tile([128, 2 * 128], BF16, tag="ph0l")
                    nc.sync.dma_start(
                        lt[:].rearrange("p (k q) -> p k q", k=2),
                        emb_wT[:, 128 * m : 128 * (m + 1)].rearrange(
                            "(k p) q -> p k q", p=128
                        ),
                    )
                    rt = ph0sb.tile([128, 2 * G4], BF16, tag="ph0r")
                    nc.sync.dma_start(
                        rt[:].rearrange("p (k n) -> p k n", k=2),
                        w0x.rearrange("(k p) n -> p k n", p=128),
                    )
                    pp = ph0ps.tile([128, G4], F32, tag="ph0ps")
                    for k in range(2):
                        for n0, n1 in ((0, 512), (512, G4)):
                            nc.tensor.matmul(
                                pp[:, n0:n1],
                                lt[:, 128 * k : 128 * (k + 1)],
                                rt[:, G4 * k + n0 : G4 * k + n1],
                                start=(k == 0),
                                stop=(k == 1),
                            )
                    nc.vector.tensor_copy(pT_sb[:, G4 * m : G4 * (m + 1)], pp[:])

            # ---- phase 1: xpart = inputsT.T @ P + b0  -> DRAM [ntok, G4] f32
            xpart = dram.tile([ntok, G4], BF16, tag="xpart")
            with (
                tc.tile_pool(name="ph1sb", bufs=3) as ph1sb,
                tc.tile_pool(name="ph1ps", bufs=2, space="PSUM") as ph1ps,
            ):
                for m in range(mtok):
                    lt = ph1sb.tile([128, KTE * 128], BF16, tag="ph1l")
                    nc.sync.dma_start(
                        lt[:].rearrange("p (k q) -> p k q", k=KTE),
                        inputsT[:, 128 * m : 128 * (m + 1)].rearrange(
                            "(k p) q -> p k q", p=128
                        ),
                    )
                    pp = ph1ps.tile([128, G4], F32, tag="ph1ps")
                    for k in range(KTE):
                        for n0, n1 in ((0, 512), (512, G4)):
                            nc.tensor.matmul(
                                pp[:, n0:n1],
                                lt[:, 128 * k : 128 * (k + 1)],
                                pT_sb[:, G4 * k + n0 : G4 * k + n1],
                                start=(k == 0),
                                stop=(k == KTE - 1),
                            )
                    xf = ph1sb.tile([128, G4], BF16, tag="ph1o")
                    nc.vector.tensor_add(xf[:], pp[:], b0rep_sb[:])
                    nc.sync.dma_start(xpart[128 * m : 128 * (m + 1), :], xf[:])

            # ---- initial hidden state tiles (feature-major [128, 12*64]) ----
            def load_hT(src, tag):
                t = sb.tile([128, KT0 * B], BF16, tag=tag)
                nc.sync.dma_start(
                    t[:].rearrange("p (j b) -> p j b", j=KT0),
                    src.rearrange("(j p) b -> p j b", p=128),
                )
                return t

            h0_prev = load_hT(h0T0, "h0")
            h1_prev = load_hT(h1T0, "h1")

            with (
                tc.tile_pool(name="psA", bufs=2, space="PSUM") as psA,
                tc.tile_pool(name="psC", bufs=1, space="PSUM") as psC,
                tc.tile_pool(name="psB", bufs=1, space="PSUM") as psB,
            ):
                # PSUM banks: lin0 2x2 + lin1 2 + lout 1 + tr 1 = 8

                def gate_chain(lin_ps, c_sb, layer):
                    """sigmoid/tanh gates, c update -> h_new [64,S] f32."""
                    act = sb.tile([B, G4], F32, tag=f"act{layer}")
                    nc.scalar.activation(
                        act[:, 0 : 3 * S], lin_ps[:, 0 : 3 * S], AF.Sigmoid
                    )
                    nc.scalar.activation(
                        act[:, 3 * S : G4], lin_ps[:, 3 * S : G4], AF.Tanh
                    )
                    fc = sb.tile([B, S], F32, tag=f"fc{layer}")
                    nc.vector.tensor_mul(fc[:], act[:, 0:S], c_sb[:])
                    ig = sb.tile([B, S], F32, tag=f"ig{layer}")
                    nc.vector.tensor_mul(
                        ig[:], act[:, S : 2 * S], act[:, 3 * S : G4]
                    )
                    nc.vector.tensor_add(c_sb[:], fc[:], ig[:])
                    th = sb.tile([B, S], F32, tag=f"th{layer}")
                    nc.scalar.activation(th[:], c_sb[:], AF.Tanh)
                    hn = sb.tile([B, S], F32, tag=f"hn{layer}")
                    nc.vector.tensor_mul(hn[:], act[:, 2 * S : 3 * S], th[:])
                    return hn

                def transpose_out(hn, ag_in, layer):
                    """[64,S] f32 -> bf16 feature-major chunks -> ag_in DRAM."""
                    tp = psB.tile([128, B], F32, tag="tr")
                    nc.tensor.transpose(tp[:], hn[:, 0:128], ident_sb[:])
                    tb = sb.tile([128, B], BF16, tag=f"trc{layer}")
                    nc.vector.tensor_copy(tb[:], tp[:])
                    nc.sync.dma_start(ag_in[0:128, :], tb[:])
                    tp2 = psB.tile([64, B], F32, tag="tr")
                    nc.tensor.transpose(tp2[:], hn[:, 128:S], ident_sb[:])
                    tb2 = sb.tile([64, B], BF16, tag=f"tsc{layer}")
                    nc.vector.tensor_copy(tb2[:], tp2[:])
                    nc.sync.dma_start(ag_in[128:S, :], tb2[:])

                def vocab_proj(h_tile, t_out):
                    lo = psB.tile([B, VS], F32, tag="lout")
                    for j in range(KT0):
                        nc.tensor.matmul(
                            lo[:],
                            h_tile[:, B * j : B * (j + 1)],
                            outw_sb[:, VS * j : VS * (j + 1)],
                            start=(j == 0),
                            stop=(j == KT0 - 1),
                        )
                    lsb = sb.tile([B, VS], F32, tag="lsb")
                    nc.vector.tensor_add(lsb[:], lo[:], outbrep_sb[:])
                    nc.sync.dma_start(logits[t_out], lsb[:])

                lout_prev = None  # h1 tile of step t-1, pending vocab proj

                for t in range(t_steps):
                    # --- A: layer-0 gate matmuls (needs h0_prev) ---
                    xp = sb.tile([B, G4], BF16, tag="xp")
                    nc.sync.dma_start(xp[:], xpart[B * t : B * (t + 1), :])
                    lin0 = psA.tile([B, G4], F32, tag="lin0")
                    for n0, n1 in ((0, 512), (512, G4)):
                        nc.tensor.matmul(
                            lin0[:, n0:n1], identb_sb[:], xp[:, n0:n1],
                            start=True, stop=False,
                        )
                        for j in range(KT0):
                            nc.tensor.matmul(
                                lin0[:, n0:n1],
                                h0_prev[:, B * j : B * (j + 1)],
                                w0h_sb[:, G4 * j + n0 : G4 * j + n1],
                                start=False, stop=(j == KT0 - 1),
                            )

                    # --- B: vocab projection of step t-1 (needs h1_prev) ---
                    if lout_prev is not None:
                        vocab_proj(lout_prev, t - 1)

                    # --- C: layer-1 h-part matmuls (needs h1_prev) ---
                    lin1 = psC.tile([B, G4], F32, tag="lin1")
                    for n0, n1 in ((0, 512), (512, G4)):
                        nc.tensor.matmul(
                            lin1[:, n0:n1], identb_sb[:], b1rep_sb[:, n0:n1],
                            start=True, stop=False,
                        )
                        for j in range(KT0):
                            nc.tensor.matmul(
                                lin1[:, n0:n1],
                                h1_prev[:, B * j : B * (j + 1)],
                                w1h_sb[:, G4 * j + n0 : G4 * j + n1],
                                start=False, stop=False,
                            )

                    # --- D: layer-0 gates, AG0 ---
                    # High priority: the chain/transpose/AG-trigger path must
                    # preempt the B/C filler matmuls at engine-idle points so
                    # AG0 launches ~8us earlier (Tile pops ready instructions
                    # by priority; default priority = program order).
                    with tc.high_priority(offset=70):
                        h0n = gate_chain(lin0, c0_sb, 0)
                        ag0_in = ring.tile([S, B], BF16, tag="ag0i")
                        transpose_out(h0n, ag0_in, 0)
                        ag0_out = ring.tile([NN, B], BF16, tag="ag0o")
                        if use_coll:
                            nc.gpsimd.collective_compute(
                                "AllGather", mybir.AluOpType.bypass,
                                replica_groups=rg,
                                ins=[ag0_in.opt()], outs=[ag0_out.opt()],
                            )
                        else:
                            nc.sync.dma_start(ag0_out[0:S, :], ag0_in[:])
                        h0_cur = load_hT(ag0_out[:], "h0")

                    # --- E: layer-1 x-part matmuls (needs h0_cur) ---
                    for n0, n1 in ((0, 512), (512, G4)):
                        for j in range(KT0):
                            nc.tensor.matmul(
                                lin1[:, n0:n1],
                                h0_cur[:, B * j : B * (j + 1)],
                                w1x_sb[:, G4 * j + n0 : G4 * j + n1],
                                start=False, stop=(j == KT0 - 1),
                            )

                    # --- F: layer-1 gates, AG1 ---
                    with tc.high_priority(offset=70):
                        h1n = gate_chain(lin1, c1_sb, 1)
                        ag1_in = ring.tile([S, B], BF16, tag="ag1i")
                        transpose_out(h1n, ag1_in, 1)
                        ag1_out = ring.tile([NN, B], BF16, tag="ag1o")
                        if use_coll:
                            nc.gpsimd.collective_compute(
                                "AllGather", mybir.AluOpType.bypass,
                                replica_groups=rg,
                                ins=[ag1_in.opt()], outs=[ag1_out.opt()],
                            )
                        else:
                            nc.sync.dma_start(ag1_out[0:S, :], ag1_in[:])
                        h1_cur = load_hT(ag1_out[:], "h1")

                    h0_prev, h1_prev, lout_prev = h0_cur, h1_cur, h1_cur

                # tail: vocab projection for the last step
                vocab_proj(lout_prev, t_steps - 1)

    _split_sync_waits(nc)
    return nc


_PROGRAM_CACHE = {}


def _get_program(t_steps=T, use_coll=True):
    key = (t_steps, use_coll)
    if key not in _PROGRAM_CACHE:
        _PROGRAM_CACHE[key] = build_program(t_steps, use_coll)
    return _PROGRAM_CACHE[key]


def make_in_maps(inputs, emb_w, lstm_w0, lstm_b0, lstm_w1, lstm_b1,
                 out_w, out_b, h0, c0, h1, c1, t_steps=T):
    bf16 = ml_dtypes.bfloat16
    f32 = np.float32
    ntok = t_steps * B
    inputsT = np.ascontiguousarray(inputs.reshape(ntok, V).T).astype(bf16)
    emb_wT = np.ascontiguousarray(emb_w.T).astype(bf16)
    ident = np.eye(64, dtype=f32)
    identb = np.eye(64).astype(bf16)

    def gate_cols(w, k):
        # [in, 4*NN] -> per-core [in, 4*S] with [f|i|o|g] blocks
        return np.concatenate(
            [w[:, g * NN + k * S : g * NN + (k + 1) * S] for g in range(4)],
            axis=1,
        )

    in_maps = []
    for k in range(NCORES):
        w0k = gate_cols(lstm_w0, k)
        w1k = gate_cols(lstm_w1, k)
        b0k = gate_cols(lstm_b0[None, :], k)[0]
        b1k = gate_cols(lstm_b1[None, :], k)[0]
        in_maps.append({
            "inputsT": inputsT,
            "emb_wT": emb_wT,
            "w0x": np.ascontiguousarray(w0k[:E]).astype(bf16),
            "w0h": np.ascontiguousarray(w0k[E:]).astype(bf16),
            "w1x": np.ascontiguousarray(w1k[:NN]).astype(bf16),
            "w1h": np.ascontiguousarray(w1k[NN:]).astype(bf16),
            "outw": np.ascontiguousarray(
                out_w[:, k * VS : (k + 1) * VS]
            ).astype(bf16),
            "b0rep": np.broadcast_to(b0k.astype(f32), (128, G4)).copy(),
            "b1rep": np.broadcast_to(b1k, (64, G4)).astype(bf16).copy(),
            "outbrep": np.broadcast_to(
                out_b[k * VS : (k + 1) * VS].astype(f32), (64, VS)
            ).copy(),
            "h0T0": np.ascontiguousarray(h0.T).astype(bf16),
            "h1T0": np.ascontiguousarray(h1.T).astype(bf16),
            "c0in": np.ascontiguousarray(c0[:, k * S : (k + 1) * S]).astype(f32),
            "c1in": np.ascontiguousarray(c1[:, k * S : (k + 1) * S]).astype(f32),
            "ident": ident,
            "identb": identb,
        })
    return in_maps


def kernel(inputs, emb_w, lstm_w0, lstm_b0, lstm_w1, lstm_b1,
           out_w, out_b, h0, c0, h1, c1, _trace=False):
    inputs = np.asarray(inputs, dtype=np.float32)
    t_steps = inputs.shape[0]
    nc = _get_program(t_steps)
    in_maps = make_in_maps(
        inputs,
        np.asarray(emb_w, np.float32), np.asarray(lstm_w0, np.float32),
        np.asarray(lstm_b0, np.float32), np.asarray(lstm_w1, np.float32),
        np.asarray(lstm_b1, np.float32), np.asarray(out_w, np.float32),
        np.asarray(out_b, np.float32), np.asarray(h0, np.float32),
        np.asarray(c0, np.float32), np.asarray(h1, np.float32),
        np.asarray(c1, np.float32), t_steps=t_steps,
    )
    res = bass_utils.run_bass_kernel_spmd(
        nc, in_maps, core_ids=list(range(NCORES)), trace=_trace
    )
    out = np.concatenate(
        [res.results[k]["logits"] for k in range(NCORES)], axis=2
    )
    if _trace:
        kernel.last_results = res
    return out.astype(np.float32)

